# revision 13
# baseline (speedup 1.0000x reference)
"""Conditional (MoE-gated) multi-head attention kernel for Trainium2.

Data-parallel over batch: 16 samples -> 8 cores x 2 samples.
Per core:
  WcT[p,s] = sum_e g[s,e] * W[p,e].T        (DVE scalar_tensor_tensor chain, fp16)
  Q.T/K.T = WcT.T-matmuls (feature-major), V natural  (PE, fp16, fp32 accum)
  per head: S = Q K^T / 8 -> exp (ACT, fused scale + row-sum) -> normalize (DVE)
            -> PE-transpose attn -> ctx^T = V^T @ attn^T -> out proj (PE)
Biases are folded in as K=1 augmented matmuls with a ones row.
"""

import numpy as np

import concourse.bass as bass
import concourse.tile as tile
from concourse import bacc
from concourse import masks, mybir
from concourse.bass_utils import run_bass_kernel_spmd

B, N, D = 16, 512, 768
H, E = 12, 8
HD = D // H  # 64
NCORES = 8
BL = B // NCORES  # samples per core
NT = N // 128  # 4 n-tiles
DT = D // 128  # 6 feature tiles
HPT = 128 // HD  # heads per 128-partition tile (2)

F16 = mybir.dt.float16
F32 = mybir.dt.float32

_BUILT = {}


def build_bass():
    nc = bacc.Bacc()

    xT_d = nc.declare_dram_parameter("xT", [BL, D, N], F16, isOutput=False)
    g_d = nc.declare_dram_parameter("g", [128, BL * E], F32, isOutput=False)
    gc_d = nc.declare_dram_parameter("gcol", [E, BL], F16, isOutput=False)
    wT_d = nc.declare_dram_parameter("wT", [3, E, D, D], F16, isOutput=False)
    bcat_d = nc.declare_dram_parameter("bcat", [E, 3 * D], F16, isOutput=False)
    woT_d = nc.declare_dram_parameter("woT", [D, D], F16, isOutput=False)
    bo_d = nc.declare_dram_parameter("bo", [1, D], F16, isOutput=False)
    out_d = nc.declare_dram_parameter("out", [BL, N, D], F32, isOutput=True)
    attn_d = nc.declare_dram_parameter("attn", [BL, H, N, N], F16, isOutput=True)

    with tile.TileContext(nc) as tc:
        with (
            tc.tile_pool(name="const", bufs=1) as cpool,
            tc.tile_pool(name="xw", bufs=1) as xwpool,
            tc.tile_pool(name="wstage", bufs=3) as wspool,
            tc.tile_pool(name="wc", bufs=2 * BL) as wcpool,
            tc.tile_pool(name="qkv", bufs=1) as qkvpool,
            tc.tile_pool(name="att", bufs=3) as apool,
            tc.tile_pool(name="attT", bufs=2) as atpool,
            tc.tile_pool(name="stats", bufs=6) as stpool,
            tc.tile_pool(name="ostage", bufs=2) as opool,
            tc.tile_pool(name="ps_mm", bufs=2, space="PSUM") as ps_mm,
            tc.tile_pool(name="ps_tp", bufs=1, space="PSUM") as ps_tp,
            tc.tile_pool(name="ps_ctx", bufs=2, space="PSUM") as ps_ctx,
        ):
            # ---- constants ----
            ident = cpool.tile([128, 128], F16, tag="ident")
            masks.make_identity(nc, ident[:])
            ones = cpool.tile([1, 512], F16, tag="ones")
            nc.gpsimd.memset(ones[:], 1.0)

            g_bc = cpool.tile([128, BL * E], F32, tag="gbc")
            nc.sync.dma_start(g_bc[:], g_d[:])

            g_col = cpool.tile([E, BL], F16, tag="gcol")
            nc.sync.dma_start(g_col[:], gc_d[:])
            bcat_sb = cpool.tile([E, 3 * D], F16, tag="bcat")
            nc.sync.dma_start(bcat_sb[:], bcat_d[:])

            woT_sb = cpool.tile([128, DT, D], F16, tag="wot")
            nc.sync.dma_start(woT_sb[:], woT_d[:].rearrange("(j p) o -> p j o", p=128))
            bo_sb = cpool.tile([1, D], F16, tag="bo")
            nc.sync.dma_start(bo_sb[:], bo_d[:])

            # ---- combined biases, per sample: bc[s] = g[s] @ bcat ----
            bc_rows = []
            for s in range(BL):
                br = cpool.tile([1, 3 * D], F16, tag=f"bcrow{s}", name=f"bcrow{s}")
                for c in range(0, 3 * D, 512):
                    w = min(512, 3 * D - c)
                    bc_ps = ps_mm.tile([1, w], F32, tag="mm", name="bc_ps")
                    nc.tensor.matmul(
                        bc_ps[:], g_col[:, s : s + 1],
                        bcat_sb[:, c : c + w], start=True, stop=True,
                    )
                    nc.scalar.copy(br[:, c : c + w], bc_ps[:])
                bc_rows.append(br)

            # ---- x transposed, per sample: [128, DT, N] ----
            xsb = []
            for s in range(BL):
                t = xwpool.tile([128, DT, N], F16, tag=f"x{s}")
                nc.sync.dma_start(t[:], xT_d[s].rearrange("(j p) n -> p j n", p=128))
                xsb.append(t)

            # ---- per projection: combine + projection matmuls ----
            qt_sb = [qkvpool.tile([128, DT, N], F16, tag=f"qt{s}", name=f"qt{s}") for s in range(BL)]
            kt_sb = [qkvpool.tile([128, DT, N], F16, tag=f"kt{s}", name=f"kt{s}") for s in range(BL)]
            v_sb = [qkvpool.tile([128, NT, D], F16, tag=f"v{s}", name=f"v{s}") for s in range(BL)]

            for p in range(3):
                wst = []
                for e in range(E):
                    t = wspool.tile([128, DT, D], F16, tag="wst")
                    nc.sync.dma_start(
                        t[:], wT_d[p, e].rearrange("(j p) o -> p j o", p=128)
                    )
                    wst.append(t)
                for s in range(BL):
                    wc = wcpool.tile([128, DT, D], F16, tag="wc")
                    nc.vector.tensor_scalar_mul(
                        wc[:], wst[0][:], g_bc[:, s * E : s * E + 1]
                    )
                    for e in range(1, E):
                        nc.vector.scalar_tensor_tensor(
                            wc[:], wst[e][:], g_bc[:, s * E + e : s * E + e + 1],
                            wc[:], mybir.AluOpType.mult, mybir.AluOpType.add,
                        )
                    if p < 2:
                        # Q.T / K.T : [o_tile 128, n 512] = WcT_tile.T @ xT
                        dst = qt_sb[s] if p == 0 else kt_sb[s]
                        for j in range(DT):  # output feature tile
                            mm = ps_mm.tile([128, N], F32, tag="mm", name="mm")
                            for it in range(DT):  # contraction tile
                                nc.tensor.matmul(
                                    mm[:],
                                    wc[:, it, j * 128 : (j + 1) * 128],
                                    xsb[s][:, it, :],
                                    start=(it == 0), stop=False,
                                )
                            nc.tensor.matmul(
                                mm[:],
                                bc_rows[s][:, p * D + j * 128 : p * D + (j + 1) * 128],
                                ones[:, 0:N],
                                start=False, stop=True,
                            )
                            nc.scalar.copy(dst[:, j, :], mm[:])
                    else:
                        # V natural: [n_tile 128, o 768]
                        for nt in range(NT):
                            for oh in range(2):
                                ow = D // 2  # 384
                                mm = ps_mm.tile([128, ow], F32, tag="mm", name="mmv")
                                for it in range(DT):
                                    nc.tensor.matmul(
                                        mm[:],
                                        xsb[s][:, it, nt * 128 : (nt + 1) * 128],
                                        wc[:, it, oh * ow : (oh + 1) * ow],
                                        start=(it == 0), stop=False,
                                    )
                                nc.tensor.matmul(
                                    mm[:],
                                    ones[:, 0:128],
                                    bc_rows[s][:, 2 * D + oh * ow : 2 * D + (oh + 1) * ow],
                                    start=False, stop=True,
                                )
                                nc.scalar.copy(
                                    v_sb[s][:, nt, oh * ow : (oh + 1) * ow], mm[:]
                                )

            # ---- attention ----
            ctxT = [qkvpool.tile([128, DT, N], F16, tag=f"ctx{s}", name=f"ctx{s}") for s in range(BL)]
            for s in range(BL):
                for h in range(H):
                    j = h // HPT
                    po = (h % HPT) * HD
                    atT = atpool.tile([128, NT, N], F16, tag="atT")
                    tps = [
                        ps_tp.tile([128, N], F16, tag=f"tp{mc}", name=f"tp{mc}") for mc in range(NT)
                    ]
                    for nt in range(NT):
                        sps = ps_mm.tile([128, N], F32, tag="mm", name="sc")
                        nc.tensor.matmul(
                            sps[:],
                            qt_sb[s][po : po + HD, j, nt * 128 : (nt + 1) * 128],
                            kt_sb[s][po : po + HD, j, :],
                            start=True, stop=True,
                        )
                        expS = apool.tile([128, N], F16, tag="expS")
                        den = stpool.tile([128, 1], F32, tag="den")
                        nc.scalar.activation(
                            expS[:], sps[:], mybir.ActivationFunctionType.Exp,
                            scale=float(1.0 / np.sqrt(HD)), accum_out=den[:],
                        )
                        rec = stpool.tile([128, 1], F32, tag="rec")
                        nc.vector.reciprocal(rec[:], den[:])
                        attn_t = apool.tile([128, N], F16, tag="attn_t")
                        nc.vector.tensor_scalar_mul(attn_t[:], expS[:], rec[:])
                        nc.sync.dma_start(
                            attn_d[s, h, nt * 128 : (nt + 1) * 128, :], attn_t[:]
                        )
                        for mc in range(NT):
                            nc.tensor.transpose(
                                tps[mc][:, nt * 128 : (nt + 1) * 128],
                                attn_t[:, mc * 128 : (mc + 1) * 128],
                                ident[:],
                            )
                    for mc in range(NT):
                        nc.scalar.copy(atT[:, mc, :], tps[mc][:])
                    cps = ps_ctx.tile([HD, N], F32, tag="cps")
                    for mc in range(NT):
                        nc.tensor.matmul(
                            cps[:],
                            v_sb[s][:, mc, h * HD : (h + 1) * HD],
                            atT[:, mc, :],
                            start=(mc == 0), stop=(mc == NT - 1),
                        )
                    nc.scalar.copy(ctxT[s][po : po + HD, j, :], cps[:])

                # ---- output projection for sample s ----
                for nt in range(NT):
                    ost = opool.tile([128, D], F32, tag="ost")
                    for oh in range(2):
                        ow = D // 2
                        mm = ps_mm.tile([128, ow], F32, tag="mm", name="mmo")
                        for it in range(DT):
                            nc.tensor.matmul(
                                mm[:],
                                ctxT[s][:, it, nt * 128 : (nt + 1) * 128],
                                woT_sb[:, it, oh * ow : (oh + 1) * ow],
                                start=(it == 0), stop=False,
                            )
                        nc.tensor.matmul(
                            mm[:],
                            ones[:, 0:128],
                            bo_sb[:, oh * ow : (oh + 1) * ow],
                            start=False, stop=True,
                        )
                        nc.scalar.copy(ost[:, oh * ow : (oh + 1) * ow], mm[:])
                    nc.sync.dma_start(out_d[s, nt * 128 : (nt + 1) * 128, :], ost[:])

    nc.finalize()
    return nc


def _get_nc():
    if "nc" not in _BUILT:
        _BUILT["nc"] = build_bass()
    return _BUILT["nc"]


def prepare_inputs(x, gating_weights, Wq, bq, Wk, bk, Wv, bv, Wo, bo):
    x = np.asarray(x, np.float32)
    g = np.asarray(gating_weights, np.float32)
    wT = np.stack(
        [
            np.ascontiguousarray(np.transpose(np.asarray(w, np.float32), (0, 2, 1)))
            for w in (Wq, Wk, Wv)
        ]
    ).astype(np.float16)
    bcat = np.concatenate(
        [np.asarray(b, np.float32) for b in (bq, bk, bv)], axis=1
    ).astype(np.float16)
    woT = np.ascontiguousarray(np.asarray(Wo, np.float32).T).astype(np.float16)
    bo_r = np.asarray(bo, np.float32).reshape(1, D).astype(np.float16)

    in_maps = []
    for c in range(NCORES):
        s0 = c * BL
        xT_c = np.ascontiguousarray(
            np.transpose(x[s0 : s0 + BL], (0, 2, 1))
        ).astype(np.float16)
        g_c = np.ascontiguousarray(g[s0 : s0 + BL])
        in_maps.append(
            {
                "xT": xT_c,
                "g": np.ascontiguousarray(
                    np.broadcast_to(g_c.reshape(1, BL * E), (128, BL * E))
                ),
                "gcol": np.ascontiguousarray(g_c.T).astype(np.float16),
                "wT": wT,
                "bcat": bcat,
                "woT": woT,
                "bo": bo_r,
            }
        )
    return in_maps


def run(in_maps, trace=False, **kw):
    nc = _get_nc()
    res = run_bass_kernel_spmd(nc, in_maps, list(range(NCORES)), trace=trace, **kw)
    out = np.empty((B, N, D), np.float32)
    attn = np.empty((B, H, N, N), np.float32)
    for c in range(NCORES):
        s0 = c * BL
        out[s0 : s0 + BL] = res.results[c]["out"]
        attn[s0 : s0 + BL] = res.results[c]["attn"].astype(np.float32)
    return (out, attn), res


def kernel(**inputs):
    in_maps = prepare_inputs(**inputs)
    (out, attn), _ = run(in_maps)
    return (out, attn)


# revision 30
# speedup vs baseline: 17.7045x; 17.7045x over previous
"""Conditional (MoE-gated) multi-head attention kernel for Trainium2.

Data-parallel over batch: 16 samples -> 8 cores x 2 samples.
Per core:
  WcT[p,s] = sum_e g[s,e] * W[p,e].T        (DVE scalar_tensor_tensor chain, fp16)
  Q.T/K.T = WcT.T-matmuls (feature-major), V natural  (PE, fp16, fp32 accum)
  per head: S = Q K^T / 8 -> exp (ACT, fused scale + row-sum) -> normalize (DVE)
            -> PE-transpose attn -> ctx^T = V^T @ attn^T -> out proj (PE)
Biases are folded in as K=1 augmented matmuls with a ones row.
"""

import numpy as np

import concourse.bass as bass
import concourse.tile as tile
from concourse import bacc
from concourse import masks, mybir
from concourse.bass_utils import run_bass_kernel_spmd

B, N, D = 16, 512, 768
H, E = 12, 8
HD = D // H  # 64
NCORES = 8
BL = B // NCORES  # samples per core
NT = N // 128  # 4 n-tiles
DT = D // 128  # 6 feature tiles
HPT = 128 // HD  # heads per 128-partition tile (2)

F16 = mybir.dt.float16
F32 = mybir.dt.float32

_BUILT = {}


def build_bass(reps=1):
    nc = bacc.Bacc()

    xT_d = nc.declare_dram_parameter("xT", [BL, D, N], F16, isOutput=False)
    g_d = nc.declare_dram_parameter("g", [128, BL * E], F32, isOutput=False)
    gc_d = nc.declare_dram_parameter("gcol", [E, BL], F16, isOutput=False)
    wT_d = nc.declare_dram_parameter("wT", [3, E, D, D], F16, isOutput=False)
    bcat_d = nc.declare_dram_parameter("bcat", [E, 3 * D], F16, isOutput=False)
    woT_d = nc.declare_dram_parameter("woT", [D, D], F16, isOutput=False)
    bo_d = nc.declare_dram_parameter("bo", [1, D], F16, isOutput=False)
    out_d = nc.declare_dram_parameter("out", [BL, N, D], F32, isOutput=True)
    attn_d = nc.declare_dram_parameter("attn", [BL, H, N, N], F16, isOutput=True)

    with tile.TileContext(nc) as tc:
        with (
            tc.tile_pool(name="const", bufs=1) as cpool,
            tc.tile_pool(name="xw", bufs=1) as xwpool,
            tc.tile_pool(name="wstage", bufs=4) as wspool,
            tc.tile_pool(name="ctmp", bufs=2) as ctpool,
            tc.tile_pool(name="wc", bufs=2 * BL) as wcpool,
            tc.tile_pool(name="qkv", bufs=1) as qkvpool,
            tc.tile_pool(name="att", bufs=2) as apool,
            tc.tile_pool(name="attT", bufs=2) as atpool,
            tc.tile_pool(name="stats", bufs=6) as stpool,
            tc.tile_pool(name="ostage", bufs=2) as opool,
            tc.tile_pool(name="ps_mm", bufs=2, space="PSUM") as ps_mm,
            tc.tile_pool(name="ps_tp", bufs=2, space="PSUM") as ps_tp,
            tc.tile_pool(name="ps_sc", bufs=2, space="PSUM") as ps_sc,
            tc.tile_pool(name="ps_ctx", bufs=2, space="PSUM") as ps_ctx,
        ):
            rep_cm = tc.For_i(0, reps, 1) if reps > 1 else None
            if rep_cm is not None:
                rep_cm.__enter__()
            # ---- constants ----
            ident = cpool.tile([128, 128], F16, tag="ident")
            masks.make_identity(nc, ident[:])
            ones = cpool.tile([1, 512], F16, tag="ones")
            nc.gpsimd.memset(ones[:], 1.0)

            g_bc = cpool.tile([128, BL * E], F32, tag="gbc")
            nc.sync.dma_start(g_bc[:], g_d[:])

            g_col = cpool.tile([E, BL], F16, tag="gcol")
            nc.sync.dma_start(g_col[:], gc_d[:])
            bcat_sb = cpool.tile([E, 3 * D], F16, tag="bcat")
            nc.sync.dma_start(bcat_sb[:], bcat_d[:])

            woT_sb = cpool.tile([128, DT, D], F16, tag="wot")
            nc.sync.dma_start(woT_sb[:], woT_d[:].rearrange("(j p) o -> p j o", p=128))
            bo_sb = cpool.tile([1, D], F16, tag="bo")
            nc.sync.dma_start(bo_sb[:], bo_d[:])

            # ---- combined biases, per sample: bc[s] = g[s] @ bcat ----
            bc_rows = []
            for s in range(BL):
                br = cpool.tile([1, 3 * D], F16, tag=f"bcrow{s}", name=f"bcrow{s}")
                for c in range(0, 3 * D, 512):
                    w = min(512, 3 * D - c)
                    bc_ps = ps_mm.tile([1, w], F32, tag="mm", name="bc_ps")
                    nc.tensor.matmul(
                        bc_ps[:], g_col[:, s : s + 1],
                        bcat_sb[:, c : c + w], start=True, stop=True,
                    )
                    nc.scalar.copy(br[:, c : c + w], bc_ps[:])
                bc_rows.append(br)

            # ---- x transposed, per sample: [128, DT, N] ----
            xsb = []
            for s in range(BL):
                t = xwpool.tile([128, DT, N], F16, tag=f"x{s}")
                nc.sync.dma_start(t[:], xT_d[s].rearrange("(j p) n -> p j n", p=128))
                xsb.append(t)

            # ---- per projection: combine + projection matmuls ----
            qt_sb = [qkvpool.tile([128, DT, N], F16, tag=f"qt{s}", name=f"qt{s}") for s in range(BL)]
            kt_sb = [qkvpool.tile([128, DT, N], F16, tag=f"kt{s}", name=f"kt{s}") for s in range(BL)]
            v_sb = [qkvpool.tile([128, NT, D], F16, tag=f"v{s}", name=f"v{s}") for s in range(BL)]

            for p in range(3):
                wst = []
                for e in range(E):
                    t = wspool.tile([128, DT, D], F16, tag="wst")
                    nc.sync.dma_start(
                        t[:], wT_d[p, e].rearrange("(j p) o -> p j o", p=128)
                    )
                    wst.append(t)
                wcs = [
                    wcpool.tile([128, DT, D], F16, tag="wc", name=f"wc{p}_{s}")
                    for s in range(BL)
                ]
                for s in range(BL):
                    nc.vector.tensor_scalar_mul(
                        wcs[s][:], wst[0][:], g_bc[:, s * E : s * E + 1]
                    )
                for e in range(1, E):
                    for s in range(BL):
                        tmp = ctpool.tile([128, DT, D], F16, tag="ctmp", name="ctmp")
                        if e % 3 == 0:
                            nc.scalar.mul(
                                tmp[:], wst[e][:], g_bc[:, s * E + e : s * E + e + 1]
                            )
                        else:
                            nc.vector.tensor_scalar_mul(
                                tmp[:], wst[e][:], g_bc[:, s * E + e : s * E + e + 1]
                            )
                        nc.vector.tensor_add(wcs[s][:], wcs[s][:], tmp[:])
                for s in range(BL):
                    wc = wcs[s]
                    if p < 2:
                        # Q.T / K.T : [o_tile 128, n 512] = WcT_tile.T @ xT
                        dst = qt_sb[s] if p == 0 else kt_sb[s]
                        for j in range(DT):  # output feature tile
                            mm = ps_mm.tile([128, N], F32, tag="mm", name="mm")
                            for it in range(DT):  # contraction tile
                                nc.tensor.matmul(
                                    mm[:],
                                    wc[:, it, j * 128 : (j + 1) * 128],
                                    xsb[s][:, it, :],
                                    start=(it == 0), stop=False,
                                )
                            nc.tensor.matmul(
                                mm[:],
                                bc_rows[s][:, p * D + j * 128 : p * D + (j + 1) * 128],
                                ones[:, 0:N],
                                start=False, stop=True,
                            )
                            nc.scalar.copy(dst[:, j, :], mm[:])
                    else:
                        # V natural: [n_tile 128, o 768]
                        for nt in range(NT):
                            for oh in range(2):
                                ow = D // 2  # 384
                                mm = ps_mm.tile([128, ow], F32, tag="mm", name="mmv")
                                for it in range(DT):
                                    nc.tensor.matmul(
                                        mm[:],
                                        xsb[s][:, it, nt * 128 : (nt + 1) * 128],
                                        wc[:, it, oh * ow : (oh + 1) * ow],
                                        start=(it == 0), stop=False,
                                    )
                                nc.tensor.matmul(
                                    mm[:],
                                    ones[:, 0:128],
                                    bc_rows[s][:, 2 * D + oh * ow : 2 * D + (oh + 1) * ow],
                                    start=False, stop=True,
                                )
                                nc.scalar.copy(
                                    v_sb[s][:, nt, oh * ow : (oh + 1) * ow], mm[:]
                                )

            # ---- attention ----
            ctxT = [qkvpool.tile([128, DT, N], F16, tag=f"ctx{s}", name=f"ctx{s}") for s in range(BL)]
            for s in range(BL):
                for h in range(H):
                    j = h // HPT
                    po = (h % HPT) * HD
                    atT = atpool.tile([128, NT, N], F16, tag="atT")
                    attn_ts = []
                    for nt in range(NT):
                        sps = ps_sc.tile([128, N], F32, tag="sc", name="sc")
                        nc.tensor.matmul(
                            sps[:],
                            qt_sb[s][po : po + HD, j, nt * 128 : (nt + 1) * 128],
                            kt_sb[s][po : po + HD, j, :],
                            start=True, stop=True,
                        )
                        expS = apool.tile([128, N], F16, tag="expS")
                        den = stpool.tile([128, 1], F32, tag="den")
                        nc.scalar.activation(
                            expS[:], sps[:], mybir.ActivationFunctionType.Exp,
                            scale=float(1.0 / np.sqrt(HD)), accum_out=den[:],
                        )
                        rec = stpool.tile([128, 1], F32, tag="rec")
                        nc.vector.reciprocal(rec[:], den[:])
                        attn_t = apool.tile([128, N], F16, tag=f"attn_t{nt}", name=f"attn_t{nt}")
                        nc.vector.tensor_scalar_mul(attn_t[:], expS[:], rec[:])
                        nc.sync.dma_start(
                            attn_d[s, h, nt * 128 : (nt + 1) * 128, :], attn_t[:]
                        )
                        attn_ts.append(attn_t)
                    for mc in range(NT):
                        tp = ps_tp.tile([128, N], F16, tag="tp", name="tp")
                        for nt in range(NT):
                            nc.tensor.transpose(
                                tp[:, nt * 128 : (nt + 1) * 128],
                                attn_ts[nt][:, mc * 128 : (mc + 1) * 128],
                                ident[:],
                            )
                        nc.vector.tensor_copy(atT[:, mc, :], tp[:])
                    cps = ps_ctx.tile([HD, N], F32, tag="cps")
                    for mc in range(NT):
                        nc.tensor.matmul(
                            cps[:],
                            v_sb[s][:, mc, h * HD : (h + 1) * HD],
                            atT[:, mc, :],
                            start=(mc == 0), stop=(mc == NT - 1),
                        )
                    nc.vector.tensor_copy(ctxT[s][po : po + HD, j, :], cps[:])

                # ---- output projection for sample s ----
                for nt in range(NT):
                    ost = opool.tile([128, D], F32, tag="ost")
                    for oh in range(2):
                        ow = D // 2
                        mm = ps_mm.tile([128, ow], F32, tag="mm", name="mmo")
                        for it in range(DT):
                            nc.tensor.matmul(
                                mm[:],
                                ctxT[s][:, it, nt * 128 : (nt + 1) * 128],
                                woT_sb[:, it, oh * ow : (oh + 1) * ow],
                                start=(it == 0), stop=False,
                            )
                        nc.tensor.matmul(
                            mm[:],
                            ones[:, 0:128],
                            bo_sb[:, oh * ow : (oh + 1) * ow],
                            start=False, stop=True,
                        )
                        nc.vector.tensor_copy(ost[:, oh * ow : (oh + 1) * ow], mm[:])
                    nc.sync.dma_start(out_d[s, nt * 128 : (nt + 1) * 128, :], ost[:])

            if rep_cm is not None:
                rep_cm.__exit__(None, None, None)
    nc.finalize()
    return nc


def _get_nc():
    if "nc" not in _BUILT:
        _BUILT["nc"] = build_bass()
    return _BUILT["nc"]


def prepare_inputs(x, gating_weights, Wq, bq, Wk, bk, Wv, bv, Wo, bo):
    x = np.asarray(x, np.float32)
    g = np.asarray(gating_weights, np.float32)
    wT = np.stack(
        [
            np.ascontiguousarray(np.transpose(np.asarray(w, np.float32), (0, 2, 1)))
            for w in (Wq, Wk, Wv)
        ]
    ).astype(np.float16)
    bcat = np.concatenate(
        [np.asarray(b, np.float32) for b in (bq, bk, bv)], axis=1
    ).astype(np.float16)
    woT = np.ascontiguousarray(np.asarray(Wo, np.float32).T).astype(np.float16)
    bo_r = np.asarray(bo, np.float32).reshape(1, D).astype(np.float16)

    in_maps = []
    for c in range(NCORES):
        s0 = c * BL
        xT_c = np.ascontiguousarray(
            np.transpose(x[s0 : s0 + BL], (0, 2, 1))
        ).astype(np.float16)
        g_c = np.ascontiguousarray(g[s0 : s0 + BL])
        in_maps.append(
            {
                "xT": xT_c,
                "g": np.ascontiguousarray(
                    np.broadcast_to(g_c.reshape(1, BL * E), (128, BL * E))
                ),
                "gcol": np.ascontiguousarray(g_c.T).astype(np.float16),
                "wT": wT,
                "bcat": bcat,
                "woT": woT,
                "bo": bo_r,
            }
        )
    return in_maps


def run(in_maps, trace=False, **kw):
    nc = _get_nc()
    res = run_bass_kernel_spmd(nc, in_maps, list(range(NCORES)), trace=trace, **kw)
    out = np.empty((B, N, D), np.float32)
    attn = np.empty((B, H, N, N), np.float32)
    for c in range(NCORES):
        s0 = c * BL
        out[s0 : s0 + BL] = res.results[c]["out"]
        attn[s0 : s0 + BL] = res.results[c]["attn"].astype(np.float32)
    return (out, attn), res


def kernel(**inputs):
    in_maps = prepare_inputs(**inputs)
    (out, attn), _ = run(in_maps)
    return (out, attn)


# revision 31
# speedup vs baseline: 20.3047x; 1.1469x over previous
"""Conditional (MoE-gated) multi-head attention kernel for Trainium2.

Data-parallel over batch: 16 samples -> 8 cores x 2 samples.
Per core:
  WcT[p,s] = sum_e g[s,e] * W[p,e].T        (DVE scalar_tensor_tensor chain, fp16)
  Q.T/K.T = WcT.T-matmuls (feature-major), V natural  (PE, fp16, fp32 accum)
  per head: S = Q K^T / 8 -> exp (ACT, fused scale + row-sum) -> normalize (DVE)
            -> PE-transpose attn -> ctx^T = V^T @ attn^T -> out proj (PE)
Biases are folded in as K=1 augmented matmuls with a ones row.
"""

import numpy as np

import concourse.bass as bass
import concourse.tile as tile
from concourse import bacc
from concourse import masks, mybir
from concourse.bass_utils import run_bass_kernel_spmd

B, N, D = 16, 512, 768
H, E = 12, 8
HD = D // H  # 64
NCORES = 8
BL = B // NCORES  # samples per core
NT = N // 128  # 4 n-tiles
DT = D // 128  # 6 feature tiles
HPT = 128 // HD  # heads per 128-partition tile (2)

F16 = mybir.dt.float16
F32 = mybir.dt.float32

_BUILT = {}


def build_bass(reps=1):
    nc = bacc.Bacc()

    xT_d = nc.declare_dram_parameter("xT", [BL, D, N], F16, isOutput=False)
    g_d = nc.declare_dram_parameter("g", [128, BL * E], F32, isOutput=False)
    gc_d = nc.declare_dram_parameter("gcol", [E, BL], F16, isOutput=False)
    wT_d = nc.declare_dram_parameter("wT", [3, E, D, D], F16, isOutput=False)
    bcat_d = nc.declare_dram_parameter("bcat", [E, 3 * D], F16, isOutput=False)
    woT_d = nc.declare_dram_parameter("woT", [D, D], F16, isOutput=False)
    bo_d = nc.declare_dram_parameter("bo", [1, D], F16, isOutput=False)
    out_d = nc.declare_dram_parameter("out", [BL, N, D], F32, isOutput=True)
    attn_d = nc.declare_dram_parameter("attn", [BL, H, N, N], F16, isOutput=True)

    with tile.TileContext(nc) as tc:
        with (
            tc.tile_pool(name="const", bufs=1) as cpool,
            tc.tile_pool(name="xw", bufs=1) as xwpool,
            tc.tile_pool(name="wstage", bufs=4) as wspool,
            tc.tile_pool(name="ctmp", bufs=2) as ctpool,
            tc.tile_pool(name="wc", bufs=2 * BL) as wcpool,
            tc.tile_pool(name="qkv", bufs=1) as qkvpool,
            tc.tile_pool(name="att", bufs=2) as apool,
            tc.tile_pool(name="attT", bufs=2) as atpool,
            tc.tile_pool(name="stats", bufs=6) as stpool,
            tc.tile_pool(name="ostage", bufs=2) as opool,
            tc.tile_pool(name="ps_mm", bufs=2, space="PSUM") as ps_mm,
            tc.tile_pool(name="ps_tp", bufs=2, space="PSUM") as ps_tp,
            tc.tile_pool(name="ps_sc", bufs=2, space="PSUM") as ps_sc,
            tc.tile_pool(name="ps_ctx", bufs=2, space="PSUM") as ps_ctx,
        ):
            rep_cm = tc.For_i(0, reps, 1) if reps > 1 else None
            if rep_cm is not None:
                rep_cm.__enter__()
            # ---- constants ----
            ident = cpool.tile([128, 128], F16, tag="ident")
            masks.make_identity(nc, ident[:])
            ones = cpool.tile([1, 512], F16, tag="ones")
            nc.gpsimd.memset(ones[:], 1.0)

            g_bc = cpool.tile([128, BL * E], F32, tag="gbc")
            nc.sync.dma_start(g_bc[:], g_d[:])

            g_col = cpool.tile([E, BL], F16, tag="gcol")
            nc.sync.dma_start(g_col[:], gc_d[:])
            bcat_sb = cpool.tile([E, 3 * D], F16, tag="bcat")
            nc.sync.dma_start(bcat_sb[:], bcat_d[:])

            woT_sb = cpool.tile([128, DT, D], F16, tag="wot")
            nc.sync.dma_start(woT_sb[:], woT_d[:].rearrange("(j p) o -> p j o", p=128))
            bo_sb = cpool.tile([1, D], F16, tag="bo")
            nc.sync.dma_start(bo_sb[:], bo_d[:])

            # ---- combined biases, per sample: bc[s] = g[s] @ bcat ----
            bc_rows = []
            for s in range(BL):
                br = cpool.tile([1, 3 * D], F16, tag=f"bcrow{s}", name=f"bcrow{s}")
                for c in range(0, 3 * D, 512):
                    w = min(512, 3 * D - c)
                    bc_ps = ps_mm.tile([1, w], F32, tag="mm", name="bc_ps")
                    nc.tensor.matmul(
                        bc_ps[:], g_col[:, s : s + 1],
                        bcat_sb[:, c : c + w], start=True, stop=True,
                    )
                    nc.scalar.copy(br[:, c : c + w], bc_ps[:])
                bc_rows.append(br)

            # ---- x transposed, per sample: [128, DT, N] ----
            xsb = []
            for s in range(BL):
                t = xwpool.tile([128, DT, N], F16, tag=f"x{s}")
                nc.sync.dma_start(t[:], xT_d[s].rearrange("(j p) n -> p j n", p=128))
                xsb.append(t)

            # ---- per projection: combine + projection matmuls ----
            qt_sb = [qkvpool.tile([128, DT, N], F16, tag=f"qt{s}", name=f"qt{s}") for s in range(BL)]
            kt_sb = [qkvpool.tile([128, DT, N], F16, tag=f"kt{s}", name=f"kt{s}") for s in range(BL)]
            v_sb = [qkvpool.tile([128, NT, D], F16, tag=f"v{s}", name=f"v{s}") for s in range(BL)]

            for p in range(3):
                wst = []
                for e in range(E):
                    t = wspool.tile([128, DT, D], F16, tag="wst")
                    nc.sync.dma_start(
                        t[:], wT_d[p, e].rearrange("(j p) o -> p j o", p=128)
                    )
                    wst.append(t)
                wcs = [
                    wcpool.tile([128, DT, D], F16, tag="wc", name=f"wc{p}_{s}")
                    for s in range(BL)
                ]
                for s in range(BL):
                    nc.vector.tensor_scalar_mul(
                        wcs[s][:], wst[0][:], g_bc[:, s * E : s * E + 1]
                    )
                for e in range(1, E):
                    for s in range(BL):
                        tmp = ctpool.tile([128, DT, D], F16, tag="ctmp", name="ctmp")
                        nc.vector.tensor_scalar_mul(
                            tmp[:], wst[e][:], g_bc[:, s * E + e : s * E + e + 1]
                        )
                        nc.vector.tensor_add(wcs[s][:], wcs[s][:], tmp[:])
                for s in range(BL):
                    wc = wcs[s]
                    if p < 2:
                        # Q.T / K.T : [o_tile 128, n 512] = WcT_tile.T @ xT
                        dst = qt_sb[s] if p == 0 else kt_sb[s]
                        for j in range(DT):  # output feature tile
                            mm = ps_mm.tile([128, N], F32, tag="mm", name="mm")
                            for it in range(DT):  # contraction tile
                                nc.tensor.matmul(
                                    mm[:],
                                    wc[:, it, j * 128 : (j + 1) * 128],
                                    xsb[s][:, it, :],
                                    start=(it == 0), stop=False,
                                )
                            nc.tensor.matmul(
                                mm[:],
                                bc_rows[s][:, p * D + j * 128 : p * D + (j + 1) * 128],
                                ones[:, 0:N],
                                start=False, stop=True,
                            )
                            nc.scalar.copy(dst[:, j, :], mm[:])
                    else:
                        # V natural: [n_tile 128, o 768]
                        for nt in range(NT):
                            for oh in range(2):
                                ow = D // 2  # 384
                                mm = ps_mm.tile([128, ow], F32, tag="mm", name="mmv")
                                for it in range(DT):
                                    nc.tensor.matmul(
                                        mm[:],
                                        xsb[s][:, it, nt * 128 : (nt + 1) * 128],
                                        wc[:, it, oh * ow : (oh + 1) * ow],
                                        start=(it == 0), stop=False,
                                    )
                                nc.tensor.matmul(
                                    mm[:],
                                    ones[:, 0:128],
                                    bc_rows[s][:, 2 * D + oh * ow : 2 * D + (oh + 1) * ow],
                                    start=False, stop=True,
                                )
                                nc.scalar.copy(
                                    v_sb[s][:, nt, oh * ow : (oh + 1) * ow], mm[:]
                                )

            # ---- attention ----
            ctxT = [qkvpool.tile([128, DT, N], F16, tag=f"ctx{s}", name=f"ctx{s}") for s in range(BL)]
            for s in range(BL):
                for h in range(H):
                    j = h // HPT
                    po = (h % HPT) * HD
                    atT = atpool.tile([128, NT, N], F16, tag="atT")
                    attn_ts = []
                    for nt in range(NT):
                        sps = ps_sc.tile([128, N], F32, tag="sc", name="sc")
                        nc.tensor.matmul(
                            sps[:],
                            qt_sb[s][po : po + HD, j, nt * 128 : (nt + 1) * 128],
                            kt_sb[s][po : po + HD, j, :],
                            start=True, stop=True,
                        )
                        expS = apool.tile([128, N], F16, tag="expS")
                        den = stpool.tile([128, 1], F32, tag="den")
                        nc.scalar.activation(
                            expS[:], sps[:], mybir.ActivationFunctionType.Exp,
                            scale=float(1.0 / np.sqrt(HD)), accum_out=den[:],
                        )
                        rec = stpool.tile([128, 1], F32, tag="rec")
                        nc.vector.reciprocal(rec[:], den[:])
                        attn_t = apool.tile([128, N], F16, tag=f"attn_t{nt}", name=f"attn_t{nt}")
                        nc.vector.tensor_scalar_mul(attn_t[:], expS[:], rec[:])
                        nc.sync.dma_start(
                            attn_d[s, h, nt * 128 : (nt + 1) * 128, :], attn_t[:]
                        )
                        attn_ts.append(attn_t)
                    for mc in range(NT):
                        tp = ps_tp.tile([128, N], F16, tag="tp", name="tp")
                        for nt in range(NT):
                            nc.tensor.transpose(
                                tp[:, nt * 128 : (nt + 1) * 128],
                                attn_ts[nt][:, mc * 128 : (mc + 1) * 128],
                                ident[:],
                            )
                        nc.vector.tensor_copy(atT[:, mc, :], tp[:])
                    cps = ps_ctx.tile([HD, N], F32, tag="cps")
                    for mc in range(NT):
                        nc.tensor.matmul(
                            cps[:],
                            v_sb[s][:, mc, h * HD : (h + 1) * HD],
                            atT[:, mc, :],
                            start=(mc == 0), stop=(mc == NT - 1),
                        )
                    nc.vector.tensor_copy(ctxT[s][po : po + HD, j, :], cps[:])

                # ---- output projection for sample s ----
                for nt in range(NT):
                    ost = opool.tile([128, D], F32, tag="ost")
                    for oh in range(2):
                        ow = D // 2
                        mm = ps_mm.tile([128, ow], F32, tag="mm", name="mmo")
                        for it in range(DT):
                            nc.tensor.matmul(
                                mm[:],
                                ctxT[s][:, it, nt * 128 : (nt + 1) * 128],
                                woT_sb[:, it, oh * ow : (oh + 1) * ow],
                                start=(it == 0), stop=False,
                            )
                        nc.tensor.matmul(
                            mm[:],
                            ones[:, 0:128],
                            bo_sb[:, oh * ow : (oh + 1) * ow],
                            start=False, stop=True,
                        )
                        nc.vector.tensor_copy(ost[:, oh * ow : (oh + 1) * ow], mm[:])
                    nc.sync.dma_start(out_d[s, nt * 128 : (nt + 1) * 128, :], ost[:])

            if rep_cm is not None:
                rep_cm.__exit__(None, None, None)
    nc.finalize()
    return nc


def _get_nc():
    if "nc" not in _BUILT:
        _BUILT["nc"] = build_bass()
    return _BUILT["nc"]


def prepare_inputs(x, gating_weights, Wq, bq, Wk, bk, Wv, bv, Wo, bo):
    x = np.asarray(x, np.float32)
    g = np.asarray(gating_weights, np.float32)
    wT = np.stack(
        [
            np.ascontiguousarray(np.transpose(np.asarray(w, np.float32), (0, 2, 1)))
            for w in (Wq, Wk, Wv)
        ]
    ).astype(np.float16)
    bcat = np.concatenate(
        [np.asarray(b, np.float32) for b in (bq, bk, bv)], axis=1
    ).astype(np.float16)
    woT = np.ascontiguousarray(np.asarray(Wo, np.float32).T).astype(np.float16)
    bo_r = np.asarray(bo, np.float32).reshape(1, D).astype(np.float16)

    in_maps = []
    for c in range(NCORES):
        s0 = c * BL
        xT_c = np.ascontiguousarray(
            np.transpose(x[s0 : s0 + BL], (0, 2, 1))
        ).astype(np.float16)
        g_c = np.ascontiguousarray(g[s0 : s0 + BL])
        in_maps.append(
            {
                "xT": xT_c,
                "g": np.ascontiguousarray(
                    np.broadcast_to(g_c.reshape(1, BL * E), (128, BL * E))
                ),
                "gcol": np.ascontiguousarray(g_c.T).astype(np.float16),
                "wT": wT,
                "bcat": bcat,
                "woT": woT,
                "bo": bo_r,
            }
        )
    return in_maps


def run(in_maps, trace=False, **kw):
    nc = _get_nc()
    res = run_bass_kernel_spmd(nc, in_maps, list(range(NCORES)), trace=trace, **kw)
    out = np.empty((B, N, D), np.float32)
    attn = np.empty((B, H, N, N), np.float32)
    for c in range(NCORES):
        s0 = c * BL
        out[s0 : s0 + BL] = res.results[c]["out"]
        attn[s0 : s0 + BL] = res.results[c]["attn"].astype(np.float32)
    return (out, attn), res


def kernel(**inputs):
    in_maps = prepare_inputs(**inputs)
    (out, attn), _ = run(in_maps)
    return (out, attn)
